# revision 1
# baseline (speedup 1.0000x reference)
"""Per-row cosine similarity: out[b, n] = <a[b,n,:], b[b,n,:]> / (||a[b,n,:]|| * ||b[b,n,:]||).

Inputs a, b: [32, 2048, 1024] f32. Output: [32, 2048] f32.

Strategy: batch-shard across 8 NeuronCores (4 batches = 8192 rows per core).
Each core streams its 64 MiB through SBUF in [128 rows, 4x1024] super-tiles
(2 MiB per DMA, HWDGE). Per 128-row tile, three fused elementwise+row-sum ops:
  - dot(a,b): DVE tensor_tensor_reduce (mult + add-reduce, one instruction)
  - sum(a^2): ACT activation(Square, accum_out=...)
  - sum(b^2): alternates DVE/ACT per tile to balance engine load
Both engines stay under the DMA roofline (~358 GB/s/core), so the kernel is
memory-bound end to end. A small epilogue computes dot/sqrt(max(sa,eps)*max(sb,eps))
with a Newton-refined sqrt, transposes [128, 64] -> [64, 128] on TensorE, and
writes the 32 KiB result with one contiguous DMA.
"""

import numpy as np

import concourse.bass as bass
import concourse.bacc as bacc
import concourse.mybir as mybir
import concourse.tile as tile
from concourse.bass_utils import run_bass_kernel_spmd
from concourse.masks import make_identity

N_CORES = 8
B, N, D = 32, 2048, 1024
ROWS_PER_CORE = (B // N_CORES) * N  # 8192
P = 128
T_SUPER = 4  # row-tiles per super-tile (2 MiB per input DMA)
N_TILES = ROWS_PER_CORE // P  # 64
N_SUPER = N_TILES // T_SUPER  # 32
IO_BUFS = 4
EPS = 1e-12

_cache: dict = {}
last_results = None  # BassKernelResults of the most recent run (for test harness)


def _build() -> bass.Bass:
    if "nc" in _cache:
        return _cache["nc"]

    f32 = mybir.dt.float32
    mult = mybir.AluOpType.mult
    add = mybir.AluOpType.add

    nc = bacc.Bacc(trn_type="TRN2")
    a_d = nc.dram_tensor("a", [ROWS_PER_CORE, D], f32, kind="ExternalInput")
    b_d = nc.dram_tensor("b", [ROWS_PER_CORE, D], f32, kind="ExternalInput")
    o_d = nc.dram_tensor("o", [ROWS_PER_CORE], f32, kind="ExternalOutput")

    a_v = a_d.rearrange("(s t p) d -> s p t d", t=T_SUPER, p=P)
    b_v = b_d.rearrange("(s t p) d -> s p t d", t=T_SUPER, p=P)

    with (
        tile.TileContext(nc) as tc,
        tc.tile_pool(name="io", bufs=IO_BUFS) as io,
        tc.tile_pool(name="scr", bufs=2) as scr,
        tc.tile_pool(name="aux", bufs=1) as aux,
        tc.tile_pool(name="ps", bufs=1, space="PSUM") as ps_pool,
    ):
        # Per-row statistics, one column per 128-row tile.
        dot = aux.tile([P, N_TILES], f32)
        sa = aux.tile([P, N_TILES], f32)
        sbE = aux.tile([P, N_TILES // 2], f32)  # sum(b^2), even tiles (DVE)
        sbO = aux.tile([P, N_TILES // 2], f32)  # sum(b^2), odd tiles (ACT)

        # The fused reduce ops must write their full-size elementwise result
        # somewhere; rotating scratch tiles keep consecutive ops independent.
        # (InstTensorTensorReduce and stride-0 broadcast outputs both crash the
        # exec unit on this runtime, so: scalar_tensor_tensor + real scratch.)
        def dve_dot(in0, in1, acc):
            dve_scr = scr.tile([P, D], f32, tag="dve_scr")
            nc.vector.scalar_tensor_tensor(
                out=dve_scr,
                in0=in0,
                scalar=1.0,
                in1=in1,
                op0=mult,
                op1=mult,
                accum_out=acc,
            )

        def act_sumsq(in0, acc):
            act_scr = scr.tile([P, D], f32, tag="act_scr")
            nc.scalar.activation(
                out=act_scr,
                in_=in0,
                func=mybir.ActivationFunctionType.Square,
                accum_out=acc,
            )

        for s in range(N_SUPER):
            a_sb = io.tile([P, T_SUPER, D], f32, tag="a_sb")
            b_sb = io.tile([P, T_SUPER, D], f32, tag="b_sb")
            nc.sync.dma_start(out=a_sb, in_=a_v[s])
            nc.sync.dma_start(out=b_sb, in_=b_v[s])
            for j in range(T_SUPER):
                t = s * T_SUPER + j
                aj = a_sb[:, j, :]
                bj = b_sb[:, j, :]
                dve_dot(aj, bj, dot[:, t : t + 1])
                act_sumsq(aj, sa[:, t : t + 1])
                if j % 2 == 0:
                    dve_dot(bj, bj, sbE[:, t // 2 : t // 2 + 1])
                else:
                    act_sumsq(bj, sbO[:, t // 2 : t // 2 + 1])

        # Epilogue: out = dot / sqrt(max(sa, EPS) * max(sb, EPS)), per row.
        H = N_TILES // 2
        dotv = dot.rearrange("p (i par) -> p par i", par=2)
        sav = sa.rearrange("p (i par) -> p par i", par=2)
        outT = aux.tile([P, N_TILES], f32)
        outTv = outT.rearrange("p (i par) -> p par i", par=2)
        mA = aux.tile([P, H], f32)
        mB = aux.tile([P, H], f32)
        d2 = aux.tile([P, H], f32)
        sq = aux.tile([P, H], f32)
        rc = aux.tile([P, H], f32)
        t1 = aux.tile([P, H], f32)
        for par, sbH in ((0, sbE), (1, sbO)):
            nc.vector.tensor_scalar_max(mA, sav[:, par, :], EPS)
            nc.vector.tensor_scalar_max(mB, sbH, EPS)
            nc.vector.tensor_mul(d2, mA, mB)
            # sqrt with one Newton step: s1 = 0.5*(s + d2/s); ACT sqrt alone
            # has a loose ULP budget.
            nc.scalar.sqrt(sq, d2)
            nc.vector.reciprocal(rc, sq)
            nc.vector.tensor_mul(t1, d2, rc)
            nc.vector.tensor_add(t1, t1, sq)
            nc.vector.tensor_scalar_mul(t1, t1, 0.5)
            nc.vector.reciprocal(rc, t1)
            nc.vector.tensor_mul(outTv[:, par, :], dotv[:, par, :], rc)

        # outT[p, t] holds the result for row t*128+p. Transpose on TensorE so
        # the store is one contiguous DMA.
        ident = aux.tile([P, P], f32)
        make_identity(nc, ident)
        ps_t = ps_pool.tile([N_TILES, P], f32)
        nc.tensor.transpose(ps_t, outT, ident)
        outF = aux.tile([N_TILES, P], f32)
        nc.scalar.copy(outF, ps_t)
        nc.sync.dma_start(out=o_d.rearrange("(t p) -> t p", p=P), in_=outF)

    nc.finalize()
    _cache["nc"] = nc
    return nc


def kernel(a: np.ndarray, b: np.ndarray, trace: bool = False, **run_kwargs) -> np.ndarray:
    global last_results
    nc = _build()
    a = np.ascontiguousarray(np.asarray(a, dtype=np.float32)).reshape(
        N_CORES, ROWS_PER_CORE, D
    )
    b = np.ascontiguousarray(np.asarray(b, dtype=np.float32)).reshape(
        N_CORES, ROWS_PER_CORE, D
    )
    in_maps = [{"a": a[k], "b": b[k]} for k in range(N_CORES)]
    res = run_bass_kernel_spmd(
        nc, in_maps, core_ids=list(range(N_CORES)), trace=trace, **run_kwargs
    )
    last_results = res
    out = np.stack([res.results[k]["o"] for k in range(N_CORES)])
    return out.reshape(B, N).astype(np.float32, copy=False)

